# revision 1
# baseline (speedup 1.0000x reference)
"""Distributed brute-force KNN (retrieval) kernel for one TRN2 chip (8 NeuronCores).

Problem: queries [256,128] f32, candidates [500000,128] f32, identifiers [500000] i32,
k=100. Output: (values [256,100] f32 desc-sorted, ids [256,100] i32).

Strategy:
  - Shard candidates over N across the 8 cores (62500 each).
  - Per core: bf16 matmul (Q stationary, C^T shard streamed) -> PSUM score
    tiles [128q, 500c]. ScalarE copies each tile to SBUF f32; VectorE folds
    500->250->125 with pairwise max (each folded slot covers a group of 4
    candidates), then max/max_index extract the top-8 (value, slot) per
    125-slot window per query-half. Claims accumulate in SBUF, one DMA out.
  - Host: expand each claimed slot to its 4 candidates, rescore contenders
    exactly in f64, and validate: any window whose 8th claimed value (or a
    duplicated claimed slot) could still hide a top-k element is fully
    rescanned on host. Exactness never depends on device numerics.
"""
import numpy as np
import ml_dtypes

B = 256          # queries
N = 500000       # candidates
D = 128          # dim
NCORES = 8
NSH = N // NCORES          # 62500 real candidates per core
TILE = 500                 # candidates per psum tile
NTILES = 125               # 62500/500
NSHP = NTILES * TILE       # == NSH (no padding needed)
FOLD = 4                   # candidates per claimed slot (two pairwise folds)
SLOTS = TILE // FOLD       # 125 slots per tile window
CLAIM = NTILES * 8         # claimed entries per (core, query-half) window row

_CACHE = {}


def build(ntiles=NTILES, loops=1, variant="fold3"):
    """Build + compile the per-core Bass program. Returns the compiled Bacc."""
    import concourse.bass as bass
    import concourse.tile as tile
    from concourse import bacc, mybir

    bf16 = mybir.dt.bfloat16
    f32 = mybir.dt.float32
    u16 = mybir.dt.uint16
    Copy = mybir.ActivationFunctionType.Copy
    nsh = ntiles * TILE

    nc = bacc.Bacc("TRN2", debug=False)
    qt = nc.dram_tensor("qt", [D, B], bf16, kind="ExternalInput").ap()
    ct = nc.dram_tensor("ct", [D, nsh], bf16, kind="ExternalInput").ap()
    v8 = nc.dram_tensor("v8", [B, ntiles * 8], f32, kind="ExternalOutput").ap()
    i8 = nc.dram_tensor("i8", [B, ntiles * 8], u16, kind="ExternalOutput").ap()

    CHUNK = 4  # ct tiles per DMA
    with tile.TileContext(nc) as tc:
        with (
            tc.tile_pool(name="qpool", bufs=1) as qpool,
            tc.tile_pool(name="cpool", bufs=3) as cpool,
            tc.tile_pool(name="psum", bufs=8, space="PSUM") as pp,
            tc.tile_pool(name="fold", bufs=4) as fpool,
            tc.tile_pool(name="acc", bufs=1) as accp,
        ):
            qtile = qpool.tile([D, B], bf16)
            nc.sync.dma_start(qtile[:], qt[:])
            vacc = [
                accp.tile([128, ntiles * 8], f32, tag=f"vacc{h}", name=f"vacc{h}")
                for h in range(2)
            ]
            iacc = [
                accp.tile([128, ntiles * 8], u16, tag=f"iacc{h}", name=f"iacc{h}")
                for h in range(2)
            ]

            def body(_iv=None):
                for tt in range(0, ntiles, CHUNK):
                    nct = min(CHUNK, ntiles - tt)
                    ctile = cpool.tile([D, CHUNK * TILE], bf16, tag="ct", name="ctile")
                    nc.sync.dma_start(
                        ctile[:, 0 : nct * TILE],
                        ct[:, bass.ds(tt * TILE, nct * TILE)],
                    )
                    for j in range(nct):
                        t = tt + j
                        for h in range(2):
                            ps = pp.tile([128, TILE], f32, name="ps")
                            nc.tensor.matmul(
                                ps[:],
                                lhsT=qtile[:, bass.ds(h * 128, 128)],
                                rhs=ctile[:, bass.ds(j * TILE, TILE)],
                                start=True,
                                stop=True,
                            )
                            vout = vacc[h][:, bass.ds(t * 8, 8)]
                            iout = iacc[h][:, bass.ds(t * 8, 8)]
                            f0 = fpool.tile([128, TILE], f32, tag="f0", name="f0")
                            nc.scalar.activation(f0[:], ps[:], Copy)
                            f1 = fpool.tile([128, TILE // 2], f32, tag="f1", name="f1")
                            nc.vector.tensor_max(
                                f1[:], f0[:, bass.ds(0, 250)], f0[:, bass.ds(250, 250)]
                            )
                            f2 = fpool.tile([128, SLOTS], f32, tag="f2", name="f2")
                            nc.vector.tensor_max(
                                f2[:], f1[:, bass.ds(0, 125)], f1[:, bass.ds(125, 125)]
                            )
                            nc.vector.max(vout, f2[:])
                            nc.vector.max_index(iout, vout, f2[:])

            if loops == 1:
                body()
            else:
                with tc.For_i(0, loops, 1) as iv:
                    body(iv)

            for h in range(2):
                nc.sync.dma_start(v8[bass.ds(h * 128, 128), :], vacc[h][:])
                nc.sync.dma_start(i8[bass.ds(h * 128, 128), :], iacc[h][:])
    nc.compile()
    return nc


def _get_nc():
    if "nc" not in _CACHE:
        _CACHE["nc"] = build()
    return _CACHE["nc"]


def make_in_maps(queries, candidates):
    qt = np.ascontiguousarray(queries.T).astype(ml_dtypes.bfloat16)
    cb = candidates.astype(ml_dtypes.bfloat16)
    in_maps = []
    for c in range(NCORES):
        ct = np.zeros((D, NSHP), dtype=ml_dtypes.bfloat16)
        ct[:, :NSH] = cb[c * NSH : (c + 1) * NSH].T
        in_maps.append({"qt": qt, "ct": ct})
    return in_maps


def _device_claims(queries, candidates):
    """Run the 8-core SPMD kernel; return claimed (vals, slot base gidx) arrays."""
    from concourse.bass_utils import run_bass_kernel_spmd

    nc = _get_nc()
    in_maps = make_in_maps(queries, candidates)
    res = None
    for attempt in range(3):
        try:
            res = run_bass_kernel_spmd(nc, in_maps, core_ids=list(range(NCORES))).results
            break
        except Exception:
            if attempt == 2:
                raise
            import time as _time

            _time.sleep(2.0)
    assert res is not None
    v8 = np.stack([r["v8"] for r in res]).astype(np.float32)   # [8, B, CLAIM]
    i8 = np.stack([r["i8"] for r in res]).astype(np.int64)     # [8, B, CLAIM] slot in [0,SLOTS)
    # padded-local base index of the claimed slot (member m adds m*SLOTS):
    offs = (np.arange(CLAIM) // 8) * TILE
    lbase = i8 + offs[None, None, :]                           # local in [0, NSHP)
    return v8, i8, lbase


def _expand_local(lb):
    """Expand local slot bases [...] -> FOLD local member indices [..., FOLD]."""
    return lb[..., None] + (np.arange(FOLD) * SLOTS)[None, :]


def kernel(queries, candidates, identifiers, k):
    queries = np.asarray(queries, dtype=np.float32)
    candidates = np.asarray(candidates, dtype=np.float32)
    identifiers = np.asarray(identifiers)
    kk = int(k)

    v8, i8, lbase = _device_claims(queries, candidates)
    core_off = (np.arange(NCORES) * NSH)[:, None, None]

    # flatten claims to [B, NCORES*CLAIM]
    vals = v8.transpose(1, 0, 2).reshape(B, NCORES * CLAIM)
    lflat = lbase.transpose(1, 0, 2).reshape(B, NCORES * CLAIM)
    cflat = np.broadcast_to(
        np.arange(NCORES)[None, :, None], (B, NCORES, CLAIM)
    ).reshape(B, NCORES * CLAIM)

    q64 = queries.astype(np.float64)
    sigma = np.linalg.norm(queries, axis=1)

    def rescore_members(lb, cores, q):
        """lb: local slot bases [M], cores [M] -> exact scores + global ids."""
        mem = _expand_local(lb)                       # [M, FOLD] local padded idx
        valid = mem < NSH
        gl = mem + cores[:, None] * NSH               # global real idx (where valid)
        gl_f = np.where(valid, gl, 0)
        sv = candidates[gl_f].astype(np.float64) @ q64[q]
        sv = np.where(valid, sv, -np.inf)
        return sv.ravel(), np.where(valid, gl, -1).ravel()

    # --- preselect top-C claims per query, rescore their groups exactly ---
    C = max(2 * kk, kk + 64)
    part = np.argpartition(-vals, C, axis=1)[:, :C]
    vsel = np.take_along_axis(vals, part, 1)
    lsel = np.take_along_axis(lflat, part, 1)
    csel = np.take_along_axis(cflat, part, 1)
    mem = _expand_local(lsel)                          # [B, C, FOLD]
    valid = mem < NSH
    gsel = np.where(valid, mem + csel[..., None] * NSH, 0)
    se = np.einsum("qcd,qd->qc", candidates[gsel.reshape(B, -1)].astype(np.float64), q64)
    se = np.where(valid.reshape(B, -1), se, -np.inf)
    se_g = se.reshape(B, C, FOLD)
    # device claim error bound per query (claim ~ max over group's exact scores)
    gmax = se_g.max(2)
    finite = np.isfinite(gmax)
    delta = np.where(finite, np.abs(vsel - gmax), 0.0).max(1)
    margin = 4.0 * delta + 1e-3 * sigma

    vk = -np.partition(-se, kk - 1, axis=1)[:, kk - 1]
    thr = vk - margin

    pool_v = [se[q] for q in range(B)]
    pool_g = [np.where(valid, mem + csel[..., None] * NSH, -1)[q].ravel() for q in range(B)]

    # 1) any claimed entry above thr that wasn't rescored
    selmask = np.zeros(vals.shape, dtype=bool)
    np.put_along_axis(selmask, part, True, 1)
    need = (vals >= thr[:, None]) & ~selmask
    for q in np.nonzero(need.any(1))[0]:
        sv, gl = rescore_members(lflat[q, need[q]], cflat[q, need[q]], q)
        pool_v[q] = np.concatenate([pool_v[q], sv])
        pool_g[q] = np.concatenate([pool_g[q], gl])

    # 2) suspect windows: (a) 8th claimed value could hide an unclaimed slot,
    #    (b) duplicated claimed slot (f32/bf16 value tie collapsing groups)
    tmin = v8[:, :, 7::8]                              # [8, B, NTILES]
    sus = tmin >= (thr - margin)[None, :, None]
    iw = np.sort(i8.reshape(NCORES, B, NTILES, 8), axis=3)
    hasdup = (np.diff(iw, axis=3) == 0).any(3)         # [8, B, NTILES]
    vmax_w = v8[:, :, 0::8]
    sus |= hasdup & (vmax_w >= (thr - margin)[None, :, None])
    for q, c, t in zip(*np.nonzero(sus.transpose(1, 0, 2))):
        base = t * TILE
        hi = min(base + TILE, NSH)
        if hi <= base:
            continue
        gb = c * NSH + base
        sv = candidates[gb : c * NSH + hi].astype(np.float64) @ q64[q]
        g = np.arange(gb, c * NSH + hi, dtype=np.int64)
        pool_v[q] = np.concatenate([pool_v[q], sv])
        pool_g[q] = np.concatenate([pool_g[q], g])

    # --- final exact top-k per query (dedupe, desc value, index tiebreak) --
    out_v = np.empty((B, kk), np.float32)
    out_g = np.empty((B, kk), np.int64)
    for q in range(B):
        keep = pool_g[q] >= 0
        g, first = np.unique(pool_g[q][keep], return_index=True)
        v32 = pool_v[q][keep][first].astype(np.float32)
        assert v32.size >= kk
        order = np.lexsort((g, -v32))[:kk]
        out_v[q] = v32[order]
        out_g[q] = g[order]

    top_ids = identifiers[out_g]
    return out_v, top_ids



# revision 5
# speedup vs baseline: 1.9780x; 1.9780x over previous
"""Distributed brute-force KNN (retrieval) kernel for one TRN2 chip (8 NeuronCores).

Problem: queries [256,128] f32, candidates [500000,128] f32, identifiers [500000] i32,
k=100. Output: (values [256,100] f32 desc-sorted, ids [256,100] i32).

Strategy (v2 — group-max fold, no on-device extraction):
  - Shard candidates over N across the 8 cores (62500 each, zero-padded to
    63488 = 31 chunks x 2048).
  - Per core: bf16 matmul (Q stationary) -> PSUM f32 score chunks
    [128q, 2048c] (4 banks). Each chunk is folded by pairwise max
    (2048->1024->512->256->128) to 128 bf16 group-maxima (FOLD=16,
    member j of slot i is local col chunk*2048 + i + 128*j). Fold1 runs
    either directly on PSUM (VectorE) or after a ScalarE f32->bf16
    evacuation copy, interleaved to balance the two engines. All 31*128
    slots per query-half accumulate in SBUF; one DMA out per half.
  - Host: rescore the top-C claimed groups exactly in f64, derive the
    device claim error bound, extend the selection to every group whose
    claim could still reach top-k, and take the exact top-k. Exactness
    never depends on device numerics (groups cover ALL candidates).
"""
import numpy as np
import ml_dtypes

B = 256          # queries
N = 500000       # candidates
D = 128          # dim
NCORES = 8
NSH = N // NCORES          # 62500 real candidates per core
CHUNK = 2048               # candidates per fold unit (4 PSUM banks)
NCHUNK = 31                # chunks per core
NSHP = NCHUNK * CHUNK      # 63488 padded candidates per core
FOLD = 16                  # candidates per claimed slot
SLOTS = CHUNK // FOLD      # 128 slots per chunk
NSLOT = NCHUNK * SLOTS     # 3968 slots per (core, query)

_CACHE = {}


def build(loops=1, variant="mix", amod=12, athr=7):
    """Build + compile the per-core Bass program. Returns the compiled Bacc.

    Two evacuation flows per 2048-chunk (TT with both operands in PSUM is
    an ISA violation, so fold1 always has >=1 SBUF operand):
      A: ScalarE copies all 2048 f32 PSUM -> SBUF bf16; VectorE does 4
         bf16 folds (2048->128) at 2x.
      C: ScalarE copies cols [1024:2048] only; VectorE fold1 is a mixed
         TT (PSUM f32 x SBUF bf16 -> bf16, 1x), then 3 bf16 folds.
    variant "mix": unit u is flow A iff (u % amod) < athr (default 7/12
    ~ the ACT/DVE balance point); "allact"/"allc" force one flow.
    """
    import concourse.bass as bass
    import concourse.tile as tile
    from concourse import bacc, mybir

    bf16 = mybir.dt.bfloat16
    f32 = mybir.dt.float32
    Copy = mybir.ActivationFunctionType.Copy

    nc = bacc.Bacc("TRN2", debug=False)
    qt = nc.dram_tensor("qt", [D, B], bf16, kind="ExternalInput").ap()
    ct = nc.dram_tensor("ct", [D, NSHP], bf16, kind="ExternalInput").ap()
    v8 = nc.dram_tensor("v8", [B, NSLOT], bf16, kind="ExternalOutput").ap()

    def flow_of(u):
        if variant == "allact":
            return "A"
        if variant == "allc":
            return "C"
        return "A" if (u % amod) < athr else "C"

    with tile.TileContext(nc) as tc:
        with (
            tc.tile_pool(name="qpool", bufs=1) as qpool,
            tc.tile_pool(name="cpool", bufs=3) as cpool,
            tc.tile_pool(name="psum", bufs=2, space="PSUM") as pp,
            tc.tile_pool(name="evac", bufs=2) as epool,
            tc.tile_pool(name="fold", bufs=2) as fpool,
            tc.tile_pool(name="acc", bufs=1) as accp,
        ):
            qtile = qpool.tile([D, B], bf16)
            nc.sync.dma_start(qtile[:], qt[:])
            vacc = [
                accp.tile([128, NSLOT], bf16, tag=f"vacc{h}", name=f"vacc{h}")
                for h in range(2)
            ]

            def body(_iv=None):
                u = 0
                for c in range(NCHUNK):
                    ctile = cpool.tile([D, CHUNK], bf16, tag="ct", name="ctile")
                    nc.sync.dma_start(
                        ctile[:], ct[:, bass.ds(c * CHUNK, CHUNK)]
                    )
                    for h in range(2):
                        ps = pp.tile([128, CHUNK], f32, name="ps")
                        for j in range(4):
                            nc.tensor.matmul(
                                ps[:, bass.ds(j * 512, 512)],
                                lhsT=qtile[:, bass.ds(h * 128, 128)],
                                rhs=ctile[:, bass.ds(j * 512, 512)],
                                start=True,
                                stop=True,
                            )
                        f1 = fpool.tile([128, 1024], bf16, tag="f1", name="f1")
                        if flow_of(u) == "A":
                            sc = epool.tile([128, CHUNK], bf16, tag="sc", name="sc")
                            nc.scalar.activation(sc[:], ps[:], Copy)
                            nc.vector.tensor_max(
                                f1[:],
                                sc[:, bass.ds(0, 1024)],
                                sc[:, bass.ds(1024, 1024)],
                            )
                        else:
                            sc = epool.tile([128, 1024], bf16, tag="sc2", name="sc2")
                            nc.scalar.activation(sc[:], ps[:, bass.ds(1024, 1024)], Copy)
                            nc.vector.tensor_max(
                                f1[:], ps[:, bass.ds(0, 1024)], sc[:]
                            )
                        f2 = fpool.tile([128, 512], bf16, tag="f2", name="f2")
                        nc.vector.tensor_max(
                            f2[:], f1[:, bass.ds(0, 512)], f1[:, bass.ds(512, 512)]
                        )
                        f3 = fpool.tile([128, 256], bf16, tag="f3", name="f3")
                        nc.vector.tensor_max(
                            f3[:], f2[:, bass.ds(0, 256)], f2[:, bass.ds(256, 256)]
                        )
                        nc.vector.tensor_max(
                            vacc[h][:, bass.ds(c * SLOTS, SLOTS)],
                            f3[:, bass.ds(0, 128)],
                            f3[:, bass.ds(128, 128)],
                        )
                        u += 1

            if loops == 1:
                body()
            else:
                with tc.For_i(0, loops, 1) as iv:
                    body(iv)

            for h in range(2):
                nc.sync.dma_start(v8[bass.ds(h * 128, 128), :], vacc[h][:])
    nc.compile()
    return nc


def _get_nc():
    if "nc" not in _CACHE:
        _CACHE["nc"] = build()
    return _CACHE["nc"]


def make_in_maps(queries, candidates):
    qt = np.ascontiguousarray(queries.T).astype(ml_dtypes.bfloat16)
    cb = candidates.astype(ml_dtypes.bfloat16)
    in_maps = []
    for c in range(NCORES):
        ct = np.zeros((D, NSHP), dtype=ml_dtypes.bfloat16)
        ct[:, :NSH] = cb[c * NSH : (c + 1) * NSH].T
        in_maps.append({"qt": qt, "ct": ct})
    return in_maps


def _device_claims(queries, candidates):
    """Run the 8-core SPMD kernel; return claims [NCORES, B, NSLOT] f32."""
    from concourse.bass_utils import run_bass_kernel_spmd

    nc = _get_nc()
    in_maps = make_in_maps(queries, candidates)
    res = None
    for attempt in range(3):
        try:
            res = run_bass_kernel_spmd(nc, in_maps, core_ids=list(range(NCORES))).results
            break
        except Exception:
            if attempt == 2:
                raise
            import time as _time

            _time.sleep(2.0)
    assert res is not None
    return np.stack([r["v8"] for r in res]).astype(np.float32)


def kernel(queries, candidates, identifiers, k):
    queries = np.asarray(queries, dtype=np.float32)
    candidates = np.asarray(candidates, dtype=np.float32)
    identifiers = np.asarray(identifiers)
    kk = int(k)

    v8 = _device_claims(queries, candidates)            # [8, B, NSLOT]

    # flatten claims to [B, NCORES*NSLOT]; group g = (core, slotcol)
    vals = v8.transpose(1, 0, 2).reshape(B, NCORES * NSLOT)

    q64 = queries.astype(np.float64)
    sigma = np.linalg.norm(queries, axis=1)

    # group id -> member global candidate indices [..., FOLD] (or <0 invalid)
    def members_of(g):
        core, sl = g // NSLOT, g % NSLOT
        c, i = sl // SLOTS, sl % SLOTS
        L = (c * CHUNK + i)[..., None] + SLOTS * np.arange(FOLD)
        valid = L < NSH
        gl = L + (core * NSH)[..., None]
        return np.where(valid, gl, -1)

    def rescore(mem, qidx):
        """mem [Q, M, FOLD] global ids (-1 invalid) -> exact f64 scores."""
        Q = mem.shape[0]
        out = np.empty(mem.shape, np.float64)
        step = 64
        for s in range(0, Q, step):
            e = min(s + step, Q)
            blk = mem[s:e]
            safe = np.where(blk >= 0, blk, 0)
            sv = np.einsum(
                "qmfd,qd->qmf",
                candidates[safe].astype(np.float64),
                q64[qidx[s:e]],
            )
            out[s:e] = np.where(blk >= 0, sv, -np.inf)
        return out

    # --- preselect top-C groups per query, rescore exactly ---
    C = max(2 * kk, kk + 64)
    part = np.argpartition(-vals, C, axis=1)[:, :C]
    vsel = np.take_along_axis(vals, part, 1)
    mem = members_of(part)                              # [B, C, FOLD]
    allq = np.arange(B)
    se = rescore(mem, allq)                             # [B, C, FOLD]
    gmax = se.max(2)
    finite = np.isfinite(gmax)
    delta = np.where(finite, np.abs(vsel - gmax), 0.0).max(1)
    margin = 4.0 * delta + 1e-3 * sigma

    flat = se.reshape(B, -1)
    vk = -np.partition(-flat, kk - 1, axis=1)[:, kk - 1]
    thr = vk - margin

    pool_v = [flat[q] for q in range(B)]
    pool_g = [mem[q].reshape(-1) for q in range(B)]

    # any group above thr that wasn't rescored yet
    selmask = np.zeros(vals.shape, dtype=bool)
    np.put_along_axis(selmask, part, True, 1)
    need = (vals >= thr[:, None]) & ~selmask
    for q in np.nonzero(need.any(1))[0]:
        g = np.nonzero(need[q])[0]
        m = members_of(g)[None]                          # [1, M, FOLD]
        sv = rescore(m, np.array([q]))[0]
        pool_v[q] = np.concatenate([pool_v[q], sv.reshape(-1)])
        pool_g[q] = np.concatenate([pool_g[q], m[0].reshape(-1)])

    # --- final exact top-k per query (dedupe, desc value, index tiebreak) --
    out_v = np.empty((B, kk), np.float32)
    out_g = np.empty((B, kk), np.int64)
    for q in range(B):
        keep = pool_g[q] >= 0
        g, first = np.unique(pool_g[q][keep], return_index=True)
        v32 = pool_v[q][keep][first].astype(np.float32)
        assert v32.size >= kk
        order = np.lexsort((g, -v32))[:kk]
        out_v[q] = v32[order]
        out_g[q] = g[order]

    top_ids = identifiers[out_g]
    return out_v, top_ids


# revision 9
# speedup vs baseline: 2.4961x; 1.2619x over previous
"""Distributed brute-force KNN (retrieval) kernel for one TRN2 chip (8 NeuronCores).

Problem: queries [256,128] f32, candidates [500000,128] f32, identifiers [500000] i32,
k=100. Output: (values [256,100] f32 desc-sorted, ids [256,100] i32).

Strategy (v2 — group-max fold, no on-device extraction):
  - Shard candidates over N across the 8 cores (62500 each, zero-padded to
    63488 = 31 chunks x 2048).
  - Per core: bf16 matmul (Q stationary) -> PSUM f32 score chunks
    [128q, 2048c] (4 banks). Each chunk is folded by pairwise max down
    to 2048/FOLD bf16 group-maxima (member j of slot i is local col
    chunk*2048 + i + (2048/FOLD)*j). Fold1 mixes a PSUM operand with a
    ScalarE-evacuated SBUF operand (flows A/C balance ACT vs DVE). All
    slots per query-half accumulate in SBUF; one DMA out per half.
  - Host: rescore the top-C claimed groups exactly in f64, derive the
    device claim error bound, extend the selection to every group whose
    claim could still reach top-k, and take the exact top-k. Exactness
    never depends on device numerics (groups cover ALL candidates).
"""
import numpy as np
import ml_dtypes

B = 256          # queries
N = 500000       # candidates
D = 128          # dim
NCORES = 8
NSH = N // NCORES          # 62500 real candidates per core
CHUNK = 2048               # candidates per fold unit (4 PSUM banks)
NCHUNK = 31                # chunks per core
NSHP = NCHUNK * CHUNK      # 63488 padded candidates per core
FOLD = 4                   # candidates per claimed slot
SLOTS = CHUNK // FOLD      # 512 slots per chunk
NSLOT = NCHUNK * SLOTS     # 15872 slots per (core, query)

_CACHE = {}


def build(loops=1, variant="mix", amod=5, athr=2):
    """Build + compile the per-core Bass program. Returns the compiled Bacc.

    Two evacuation flows per 2048-chunk (TT with both operands in PSUM is
    an ISA violation, so fold1 always has >=1 SBUF operand):
      A: ScalarE copies all 2048 f32 PSUM -> SBUF bf16; VectorE does 4
         bf16 folds (2048->128) at 2x.
      C: ScalarE copies cols [1024:2048] only; VectorE fold1 is a mixed
         TT (PSUM f32 x SBUF bf16 -> bf16, 1x), then 3 bf16 folds.
    variant "mix": unit u is flow A iff (u % amod) < athr (default 7/12
    ~ the ACT/DVE balance point); "allact"/"allc" force one flow.
    """
    import concourse.bass as bass
    import concourse.tile as tile
    from concourse import bacc, mybir

    bf16 = mybir.dt.bfloat16
    f32 = mybir.dt.float32
    Copy = mybir.ActivationFunctionType.Copy

    nc = bacc.Bacc("TRN2", debug=False)
    qt = nc.dram_tensor("qt", [D, B], bf16, kind="ExternalInput").ap()
    ct = nc.dram_tensor("ct", [D, NSHP], bf16, kind="ExternalInput").ap()
    v8 = nc.dram_tensor("v8", [B, NSLOT], bf16, kind="ExternalOutput").ap()

    def flow_of(u):
        if variant == "allact":
            return "A"
        if variant == "allc":
            return "C"
        return "A" if (u % amod) < athr else "C"

    with tile.TileContext(nc) as tc:
        with (
            tc.tile_pool(name="qpool", bufs=1) as qpool,
            tc.tile_pool(name="cpool", bufs=3) as cpool,
            tc.tile_pool(name="psum", bufs=2, space="PSUM") as pp,
            tc.tile_pool(name="evac", bufs=2) as epool,
            tc.tile_pool(name="fold", bufs=2) as fpool,
            tc.tile_pool(name="acc", bufs=1) as accp,
        ):
            qtile = qpool.tile([D, B], bf16)
            nc.sync.dma_start(qtile[:], qt[:])
            vacc = [
                accp.tile([128, NSLOT], bf16, tag=f"vacc{h}", name=f"vacc{h}")
                for h in range(2)
            ]

            def body(_iv=None):
                u = 0
                for c in range(NCHUNK):
                    ctile = cpool.tile([D, CHUNK], bf16, tag="ct", name="ctile")
                    nc.sync.dma_start(
                        ctile[:], ct[:, bass.ds(c * CHUNK, CHUNK)]
                    )
                    for h in range(2):
                        ps = pp.tile([128, CHUNK], f32, name="ps")
                        for j in range(4):
                            nc.tensor.matmul(
                                ps[:, bass.ds(j * 512, 512)],
                                lhsT=qtile[:, bass.ds(h * 128, 128)],
                                rhs=ctile[:, bass.ds(j * 512, 512)],
                                start=True,
                                stop=True,
                            )
                        f1 = fpool.tile([128, 1024], bf16, tag="f1", name="f1")
                        if flow_of(u) == "A":
                            sc = epool.tile([128, CHUNK], bf16, tag="sc", name="sc")
                            nc.scalar.activation(sc[:], ps[:], Copy)
                            nc.vector.tensor_max(
                                f1[:],
                                sc[:, bass.ds(0, 1024)],
                                sc[:, bass.ds(1024, 1024)],
                            )
                        else:
                            sc = epool.tile([128, 1024], bf16, tag="sc2", name="sc2")
                            nc.scalar.activation(sc[:], ps[:, bass.ds(1024, 1024)], Copy)
                            nc.vector.tensor_max(
                                f1[:], ps[:, bass.ds(0, 1024)], sc[:]
                            )
                        # remaining bf16 folds down to SLOTS wide
                        w = 1024
                        cur = f1
                        while w // 2 > SLOTS:
                            w //= 2
                            nxt = fpool.tile([128, w], bf16, tag=f"f{w}", name=f"f{w}")
                            nc.vector.tensor_max(
                                nxt[:], cur[:, bass.ds(0, w)], cur[:, bass.ds(w, w)]
                            )
                            cur = nxt
                        nc.vector.tensor_max(
                            vacc[h][:, bass.ds(c * SLOTS, SLOTS)],
                            cur[:, bass.ds(0, SLOTS)],
                            cur[:, bass.ds(SLOTS, SLOTS)],
                        )
                        u += 1

            if loops == 1:
                body()
            else:
                with tc.For_i(0, loops, 1) as iv:
                    body(iv)

            for h in range(2):
                nc.sync.dma_start(v8[bass.ds(h * 128, 128), :], vacc[h][:])
    nc.compile()
    return nc


def _get_nc():
    if "nc" not in _CACHE:
        _CACHE["nc"] = build()
    return _CACHE["nc"]


def make_in_maps(queries, candidates):
    qt = np.ascontiguousarray(queries.T).astype(ml_dtypes.bfloat16)
    cb = candidates.astype(ml_dtypes.bfloat16)
    in_maps = []
    for c in range(NCORES):
        ct = np.zeros((D, NSHP), dtype=ml_dtypes.bfloat16)
        ct[:, :NSH] = cb[c * NSH : (c + 1) * NSH].T
        in_maps.append({"qt": qt, "ct": ct})
    return in_maps


def _device_claims(queries, candidates):
    """Run the 8-core SPMD kernel; return claims [NCORES, B, NSLOT] f32."""
    from concourse.bass_utils import run_bass_kernel_spmd

    nc = _get_nc()
    in_maps = make_in_maps(queries, candidates)
    res = None
    for attempt in range(3):
        try:
            res = run_bass_kernel_spmd(nc, in_maps, core_ids=list(range(NCORES))).results
            break
        except Exception:
            if attempt == 2:
                raise
            import time as _time

            _time.sleep(2.0)
    assert res is not None
    return np.stack([r["v8"] for r in res]).astype(np.float32)


def kernel(queries, candidates, identifiers, k):
    queries = np.asarray(queries, dtype=np.float32)
    candidates = np.asarray(candidates, dtype=np.float32)
    identifiers = np.asarray(identifiers)
    kk = int(k)

    v8 = _device_claims(queries, candidates)            # [8, B, NSLOT]

    # flatten claims to [B, NCORES*NSLOT]; group g = (core, slotcol)
    vals = v8.transpose(1, 0, 2).reshape(B, NCORES * NSLOT)

    q64 = queries.astype(np.float64)
    sigma = np.linalg.norm(queries, axis=1)

    # group id -> member global candidate indices [..., FOLD] (or <0 invalid)
    def members_of(g):
        core, sl = g // NSLOT, g % NSLOT
        c, i = sl // SLOTS, sl % SLOTS
        L = (c * CHUNK + i)[..., None] + SLOTS * np.arange(FOLD)
        valid = L < NSH
        gl = L + (core * NSH)[..., None]
        return np.where(valid, gl, -1)

    def rescore(mem, qidx):
        """mem [Q, M, FOLD] global ids (-1 invalid) -> exact f64 scores."""
        Q = mem.shape[0]
        out = np.empty(mem.shape, np.float64)
        step = 64
        for s in range(0, Q, step):
            e = min(s + step, Q)
            blk = mem[s:e]
            safe = np.where(blk >= 0, blk, 0)
            sv = np.einsum(
                "qmfd,qd->qmf",
                candidates[safe].astype(np.float64),
                q64[qidx[s:e]],
            )
            out[s:e] = np.where(blk >= 0, sv, -np.inf)
        return out

    # --- preselect top-C groups per query, rescore exactly ---
    C = max(2 * kk, kk + 64)
    part = np.argpartition(-vals, C, axis=1)[:, :C]
    vsel = np.take_along_axis(vals, part, 1)
    mem = members_of(part)                              # [B, C, FOLD]
    allq = np.arange(B)
    se = rescore(mem, allq)                             # [B, C, FOLD]
    gmax = se.max(2)
    finite = np.isfinite(gmax)
    delta = np.where(finite, np.abs(vsel - gmax), 0.0).max(1)
    margin = 4.0 * delta + 1e-3 * sigma

    flat = se.reshape(B, -1)
    vk = -np.partition(-flat, kk - 1, axis=1)[:, kk - 1]
    thr = vk - margin

    pool_v = [flat[q] for q in range(B)]
    pool_g = [mem[q].reshape(-1) for q in range(B)]

    # any group above thr that wasn't rescored yet
    selmask = np.zeros(vals.shape, dtype=bool)
    np.put_along_axis(selmask, part, True, 1)
    need = (vals >= thr[:, None]) & ~selmask
    for q in np.nonzero(need.any(1))[0]:
        g = np.nonzero(need[q])[0]
        m = members_of(g)[None]                          # [1, M, FOLD]
        sv = rescore(m, np.array([q]))[0]
        pool_v[q] = np.concatenate([pool_v[q], sv.reshape(-1)])
        pool_g[q] = np.concatenate([pool_g[q], m[0].reshape(-1)])

    # --- final exact top-k per query (dedupe, desc value, index tiebreak) --
    out_v = np.empty((B, kk), np.float32)
    out_g = np.empty((B, kk), np.int64)
    for q in range(B):
        keep = pool_g[q] >= 0
        g, first = np.unique(pool_g[q][keep], return_index=True)
        v32 = pool_v[q][keep][first].astype(np.float32)
        assert v32.size >= kk
        order = np.lexsort((g, -v32))[:kk]
        out_v[q] = v32[order]
        out_g[q] = g[order]

    top_ids = identifiers[out_g]
    return out_v, top_ids
